# revision 2
# baseline (speedup 1.0000x reference)
"""Trainium2 Bass kernel v2 for nn_Net_12481174962824 (binarized CNN).

Data-parallel over 8 cores (512 images each). Per-core structure:
  - Image pairs (j, j+32) are M-packed on partitions [img-A ch | img-B ch]
    through conv1 -> conv2 -> conv3, so no shifted activation copies exist.
  - conv1: fp16 2-term split of x on K with 9 taps (18 rows/band, block-diag
    lhsT). The quant affine (x3 + 3b+8) is folded into the matmul: weights
    are 3*sign(w), the bias enters via constant-1 rows of T1 (f16-split
    into 2 rows for precision). One [128, 392] matmul per pair-half.
  - Activations are stored as z = q + 8 in fp8e4 ({8..11}): e4m3's ulp is 1
    in [8,16), so the fp8 conversion itself performs RNE round-to-int; the
    8*sum(w) correction is folded into the next layer's bias.
  - conv2/conv3: 5 DoubleRow fp8 passes (2 taps per pass via the rhs i-dim
    stride), K = 128 partitions = the M-packed channel pairs.
  - maxpool fused with the lower clamp via scalar_tensor_tensor; upper clamp
    via tensor_scalar_min writing fp8 directly (RNE quantization).
  - fc1: 7 DoubleRow fp8 passes over a [128, 14, 512] staged feature tensor;
    fc2 fp32 with fc1 acts stationary -> logits batch-on-partitions.
"""

import numpy as np
import ml_dtypes

F16 = np.float16
FP8 = ml_dtypes.float8_e4m3fn
F32 = np.float32
N_CORES = 8
B_CORE = 512
NB = 64               # images per chunk
NCHUNK = B_CORE // NB
NPAIR = NB // 2       # M-packed pairs per chunk (32)

# conv2/conv3 tap schedule: 5 DoubleRow passes, each covering (i=0, i=1).
C2_PASSES = [
    ((0, 0), (0, 1)),
    ((0, 2), (1, 0)),
    ((1, 1), (1, 2)),
    ((2, 0), (2, 1)),
    ((2, 2), None),   # i=1 masked (weights zero)
]


def _f32(x):
    return np.asarray(x, dtype=np.float32)


def _sg(w):
    return np.where(_f32(w) >= 0, np.float32(1), np.float32(-1))


def _prep(w1, b1, w2, g1, be1, m1, v1, w3, g2, be2, m2, v2, fw1, fb1, fw2, fb2):
    # conv1 lhsT [64, 128] f16: row r = 9*s + 3*dy + dx (s in {0,1}) holds
    # 3*sign(w1); rows 18/19 (band0) and 50/51 (band1) hold the f16-split
    # quant bias (3*b1 + 8). band0 rows -> cols 0-63, band1 -> cols 64-127.
    w1b = _sg(w1)[:, 0]                       # [64, 3, 3]
    taps3 = (3.0 * w1b.reshape(64, 9).T).astype(F16)  # [9, 64]
    bias = (3.0 * _f32(b1) + 8.0).astype(F32)
    bhi = bias.astype(F16)
    blo = (bias - bhi.astype(F32)).astype(F16)
    # K layout (40 contiguous rows): band0 taps 0-17, bias 18-19;
    # band1 taps 20-37, bias 38-39.
    w1l = np.zeros((64, 128), dtype=F16)
    for s in range(2):
        w1l[9 * s: 9 * s + 9, 0:64] = taps3
        w1l[20 + 9 * s: 20 + 9 * s + 9, 64:128] = taps3
    w1l[18, 0:64] = bhi
    w1l[19, 0:64] = blo
    w1l[38, 64:128] = bhi
    w1l[39, 64:128] = blo

    def fold_inv(g, v):
        inv = (_f32(g) / np.sqrt(_f32(v) + np.float32(1e-4))).astype(F32)
        assert (inv > 0).all(), "negative BN scale breaks pool/quant commute"
        return inv

    inv1 = fold_inv(g1, v1)
    inv2 = fold_inv(g2, v2)

    def dr_weights(wb):
        out = np.zeros((128, 5, 2, 128), dtype=FP8)
        for p, (t0, t1) in enumerate(C2_PASSES):
            for i, t in enumerate((t0, t1)):
                if t is None:
                    continue
                blk = wb[:, :, t[0], t[1]].T.astype(FP8)  # [cin, cout]
                out[0:64, p, i, 0:64] = blk
                out[64:128, p, i, 64:128] = blk
        return out

    w2b, w3b = _sg(w2), _sg(w3)
    w2l = dr_weights(w2b).reshape(128, 5 * 2 * 128)
    w3l = dr_weights(w3b).reshape(128, 5 * 2 * 128)

    def rep(v):
        return np.concatenate([_f32(v), _f32(v)]).astype(F32).reshape(128, 1)

    sumw2 = w2b.sum(axis=(1, 2, 3)).astype(F32)
    sumw3 = w3b.sum(axis=(1, 2, 3)).astype(F32)
    b2c = (3 * _f32(be1) - 3 * _f32(m1) * inv1 - 8.0 * inv1 * sumw2 + 8.0)
    b3c = (3 * _f32(be2) - 3 * _f32(m2) * inv2 - 8.0 * inv2 * sumw3 + 8.0)
    cv = np.concatenate([
        rep(inv1),                   # 0: CB2s
        rep(b2c),                    # 1: CB2b
        rep((8.0 - b2c) / inv1),     # 2: LB2 (psum units)
        rep(inv2),                   # 3: CB3s
        rep(b3c),                    # 4: CB3b
        rep(np.zeros(64)), rep(np.zeros(64)), rep(np.zeros(64)),
    ], axis=1).astype(F32)           # [128, 8]

    fw1b = _sg(fw1)                  # [512, 1600]
    fw1l = np.zeros((128, 14, 512), dtype=FP8)
    for kl in range(14):
        f0 = 128 * kl
        n = min(128, 1600 - f0)
        if n > 0:
            fw1l[0:n, kl, :] = fw1b[:, f0:f0 + n].T.astype(FP8)
    fb1c = (_f32(fb1) - np.float32(8.0 / 3.0) * fw1b.sum(axis=1)).astype(F32)
    fb1v = fb1c.reshape(4, 128).T.copy()   # [128, 4]

    fw2l = np.zeros((128, 4, 10), dtype=F32)
    for k2 in range(4):
        fw2l[:, k2, :] = _f32(fw2)[:, 128 * k2:128 * (k2 + 1)].T
    fw2l = fw2l.reshape(128, 40)
    fb2v = _f32(fb2).reshape(10, 1).copy()
    return dict(w1l=w1l, w2l=w2l, w3l=w3l, cv=cv,
                fw1l=fw1l.reshape(128, 14 * 512), fw2l=fw2l,
                fb1v=fb1v, fb2v=fb2v)


def _split_x(x_shard):
    """[512,28,28] f32 -> two padded f16 split tensors [512*900]."""
    S = np.zeros((B_CORE, 30, 30), dtype=F32)
    S[:, 1:29, 1:29] = x_shard
    S = S.reshape(-1)
    t0 = S.astype(F16)
    t1 = (S - t0.astype(F32)).astype(F16)
    pad = np.zeros(64, dtype=F16)
    return np.concatenate([t0, pad]), np.concatenate([t1, pad])


def _build_nc():
    import concourse.bass as bass
    import concourse.bacc as bacc
    import concourse.tile as tile
    import concourse.mybir as mybir
    from contextlib import ExitStack

    fp32 = mybir.dt.float32
    f16 = mybir.dt.float16
    fp8 = mybir.dt.float8e4
    AF = mybir.ActivationFunctionType
    ALU = mybir.AluOpType
    DR = mybir.MatmulPerfMode.DoubleRow
    AX = mybir.AxisListType.X
    AXY = mybir.AxisListType.XY

    nc = bacc.Bacc("TRN2", target_bir_lowering=False)
    d_s0 = nc.dram_tensor("s0", [B_CORE * 900 + 64], f16, kind="ExternalInput")
    d_s1 = nc.dram_tensor("s1", [B_CORE * 900 + 64], f16, kind="ExternalInput")
    d_ones = nc.dram_tensor("ones", [NPAIR * 840], f16, kind="ExternalInput")
    d_w1 = nc.dram_tensor("w1l", [64, 128], f16, kind="ExternalInput")
    d_w2 = nc.dram_tensor("w2l", [128, 1280], fp8, kind="ExternalInput")
    d_w3 = nc.dram_tensor("w3l", [128, 1280], fp8, kind="ExternalInput")
    d_cv = nc.dram_tensor("cv", [128, 8], fp32, kind="ExternalInput")
    d_fw1 = nc.dram_tensor("fw1l", [128, 14 * 512], fp8, kind="ExternalInput")
    d_fw2 = nc.dram_tensor("fw2l", [128, 40], fp32, kind="ExternalInput")
    d_fb1 = nc.dram_tensor("fb1v", [128, 4], fp32, kind="ExternalInput")
    d_fb2 = nc.dram_tensor("fb2v", [10, 1], fp32, kind="ExternalInput")
    d_out = nc.dram_tensor("out", [B_CORE, 10], fp32, kind="ExternalOutput")

    splits = [d_s0, d_s1]
    c13 = float(np.float32(1.0) / np.float32(3.0))

    with tile.TileContext(nc) as tc, ExitStack() as ctx:
        singles = ctx.enter_context(tc.tile_pool(name="singles", bufs=1))
        tmp = ctx.enter_context(tc.tile_pool(name="tmp", bufs=3))
        ps_c1 = ctx.enter_context(tc.tile_pool(name="ps_c1", bufs=2, space="PSUM"))
        ps_c2 = ctx.enter_context(tc.tile_pool(name="ps_c2", bufs=2, space="PSUM"))

        # ---- weights / constants ----
        W1 = singles.tile([64, 128], f16)
        nc.sync.dma_start(out=W1, in_=d_w1[:, :])
        W2 = singles.tile([128, 5, 2, 128], fp8)
        nc.sync.dma_start(out=W2, in_=d_w2[:, :].rearrange("p (a i m) -> p a i m", a=5, i=2))
        W3 = singles.tile([128, 5, 2, 128], fp8)
        nc.sync.dma_start(out=W3, in_=d_w3[:, :].rearrange("p (a i m) -> p a i m", a=5, i=2))
        CV = singles.tile([128, 8], fp32)
        nc.sync.dma_start(out=CV, in_=d_cv[:, :])
        FW1 = singles.tile([128, 14, 512], fp8)
        nc.sync.dma_start(out=FW1, in_=d_fw1[:, :].rearrange("p (k m) -> p k m", k=14))
        FW2 = singles.tile([128, 4, 10], fp32)
        nc.sync.dma_start(out=FW2, in_=d_fw2[:, :].rearrange("p (k m) -> p k m", k=4))
        FB1 = singles.tile([128, 4], fp32)
        nc.sync.dma_start(out=FB1, in_=d_fb1[:, :])
        FB2T = singles.tile([128, 10], fp32)
        fb2b = bass.AP(tensor=d_fb2[:, :].tensor, offset=0, ap=[[0, 128], [1, 10]])
        nc.sync.dma_start(out=FB2T, in_=fb2b)

        # ---- persistent activation tiles ----
        T1 = [singles.tile([64, NPAIR, 840], f16, tag=f"T1{i}", name=f"T1{i}")
              for i in range(2)]
        T2 = [singles.tile([128, NPAIR * 256 + 64], fp8, tag=f"T2{i}", name=f"T2{i}")
              for i in range(2)]
        T3 = [singles.tile([128, NPAIR * 49 + 64], fp8, tag=f"T3{i}", name=f"T3{i}")
              for i in range(2)]
        Q3 = [singles.tile([128, 28 * NPAIR], fp8, tag=f"Q3{i}", name=f"Q3{i}")
              for i in range(2)]
        F = singles.tile([128, 14, 512], fp8)
        H1 = [singles.tile([128, B_CORE], fp32, tag=f"H1_{m}", name=f"H1_{m}")
              for m in range(4)]
        # fp8 "zero" activation is z=8; borders/pads must read as 8.
        # T1 rows 18/19, 50/51 are the constant-1 bias rows; 20-31/52-63 are
        # in the K range of nothing (K=52 covers 0..51) but keep them finite.
        for i in range(2):
            nc.gpsimd.memset(T2[i], 8)
            nc.gpsimd.memset(T3[i], 8)
            nc.vector.memset(Q3[i][:, 25 * NPAIR:], 8)
            for r0 in (18, 38):
                ones_src = bass.AP(tensor=d_ones[:].tensor, offset=0,
                                   ap=[[0, 2], [1, NPAIR * 840]])
                nc.sync.dma_start(out=T1[i][r0:r0 + 2, :, :], in_=ones_src)

        for ch in range(NCHUNK):
            t1 = T1[ch % 2]
            t2 = T2[ch % 2]
            t3 = T3[ch % 2]
            q3 = Q3[ch % 2]
            # ---- conv1 input loads: 12 DMAs per (band g, split s, dy) ----
            for g in range(2):
                for s in range(2):
                    for dy in range(3):
                        src = bass.AP(
                            tensor=splits[s][:].tensor,
                            offset=(ch * NB + g * NPAIR) * 900 + dy * 30,
                            ap=[[1, 3], [900, NPAIR], [1, 840]])
                        r0 = 20 * g + 9 * s + 3 * dy
                        nc.sync.dma_start(out=t1[r0:r0 + 3, :, :], in_=src)

            # ---- conv1 (affine folded in PE) + pool + quant, per pair ----
            t1v = t1.rearrange("p b (y x) -> p b y x", y=28)
            for j in range(NPAIR):
                ps = ps_c1.tile([128, 2, 512], fp32, tag="c1")
                for h in range(2):
                    nc.tensor.matmul(
                        ps[:, h, 0:392], W1[0:40, :],
                        t1v[0:40, j, 14 * h:14 * h + 14, 0:28],
                        start=True, stop=True)
                # ACT drains the odd-dx phase PSUM->SBUF; DVE TT-max pairs it
                # with the even phase (single PSUM operand is legal).
                psv = ps[:, :, 0:392].rearrange(
                    "p h (y x2 dx) -> p h y x2 dx", y=14, dx=2)
                C1 = tmp.tile([128, 2, 14, 14], fp32, tag="c1cp")
                nc.scalar.activation(out=C1, in_=psv[:, :, :, :, 1],
                                     func=AF.Copy, bias=0.0, scale=1.0)
                M1 = tmp.tile([128, 2, 14, 14], fp32, tag="m1")
                nc.vector.tensor_max(M1, psv[:, :, :, :, 0], C1)
                # dy max-pool with folded lower clamp (8.0)
                M1v = M1.rearrange("p h (y2 dy) x -> p h y2 dy x", dy=2)
                U1 = tmp.tile([128, 2, 7, 14], fp32, tag="u1")
                nc.vector.scalar_tensor_tensor(
                    out=U1, in0=M1v[:, :, :, 0, :], scalar=8.0,
                    in1=M1v[:, :, :, 1, :], op0=ALU.max, op1=ALU.max)
                # upper clamp + fp8 RNE round into T2 interior
                dst = bass.AP(
                    tensor=t2.tensor, offset=t2.offset + j * 256 + 17,
                    ap=list(t2.ap[:1]) + [[112, 2], [16, 7], [1, 14]])
                nc.vector.tensor_scalar_min(
                    out=dst, in0=U1.rearrange("p h y x -> p (h y x)"),
                    scalar1=11.0)

            # ---- conv2 + conv3 interleaved: blocks of 4 pairs ----
            # conv3 accumulates in the unused tail columns (224:250) of the
            # same PSUM tiles, so both fit in 4 banks with double buffering.
            def conv2_blk(blk):
                ps2 = ps_c2.tile([128, 4, 256], fp32, tag="c2")
                for p in range(5):
                    (dy0, dx0) = C2_PASSES[p][0]
                    o0 = dy0 * 16 + dx0
                    tp1 = C2_PASSES[p][1]
                    delta = (tp1[0] * 16 + tp1[1] - o0) if tp1 is not None else 1
                    for q in range(4):
                        j = blk * 4 + q
                        rhs = bass.AP(
                            tensor=t2.tensor, offset=t2.offset + j * 256 + o0,
                            ap=list(t2.ap[:1]) + [[delta, 2], [16, 14], [1, 14]])
                        nc.tensor.matmul(
                            ps2[:, q, 0:196], W2[:, p], rhs,
                            start=(p == 0), stop=(p == 4), perf_mode=DR)
                psv2 = ps2[:, :, 0:196].rearrange(
                    "p jj (y x2 dx) -> p jj y x2 dx", y=14, dx=2)
                C2 = tmp.tile([128, 4, 14, 7], fp32, tag="c2cp")
                nc.scalar.activation(out=C2, in_=psv2[:, :, :, :, 1],
                                     func=AF.Copy, bias=0.0, scale=1.0)
                M2 = tmp.tile([128, 4, 14, 7], fp32, tag="m2")
                nc.vector.tensor_max(M2, psv2[:, :, :, :, 0], C2)
                M2v = M2.rearrange("p m (y2 dy) x -> p m y2 dy x", dy=2)
                U2 = tmp.tile([128, 4, 7, 7], fp32, tag="u2")
                nc.vector.tensor_max(U2, M2v[:, :, :, 0, :], M2v[:, :, :, 1, :])
                Y2 = tmp.tile([128, 196], fp32, tag="y2")
                nc.scalar.activation(out=Y2, in_=U2.rearrange("p m y x -> p (m y x)"),
                                     func=AF.Identity, bias=CV[:, 1:2],
                                     scale=CV[:, 0:1])
                dst3 = bass.AP(
                    tensor=t3.tensor, offset=t3.offset + blk * 4 * 49,
                    ap=list(t3.ap[:1]) + [[1, 196]])
                nc.vector.tensor_scalar(out=dst3, in0=Y2, scalar1=8.0,
                                        scalar2=11.0, op0=ALU.max, op1=ALU.min)

            def conv3_blk(bb):
                ps3t = ps_c2.tile([128, 4, 256], fp32, tag="c2")
                ps3 = ps3t[:, :, 224:250]
                for p in range(5):
                    (dy0, dx0) = C2_PASSES[p][0]
                    o0 = dy0 * 7 + dx0
                    tp1 = C2_PASSES[p][1]
                    delta = (tp1[0] * 7 + tp1[1] - o0) if tp1 is not None else 1
                    for q in range(4):
                        j = bb * 4 + q
                        rhs = bass.AP(
                            tensor=t3.tensor, offset=t3.offset + j * 49 + o0,
                            ap=list(t3.ap[:1]) + [[delta, 2], [7, 5], [1, 5]])
                        nc.tensor.matmul(ps3[:, q, 0:25], W3[:, p], rhs,
                                         start=(p == 0), stop=(p == 4),
                                         perf_mode=DR)
                Y3 = tmp.tile([128, 4, 25], fp32, tag="y3")
                nc.scalar.activation(out=Y3, in_=ps3[:, :, 0:25],
                                     func=AF.Identity, bias=CV[:, 4:5],
                                     scale=CV[:, 3:4])
                dstq = bass.AP(
                    tensor=q3.tensor, offset=q3.offset + bb * 4,
                    ap=list(q3.ap[:1]) + [[1, 4], [NPAIR, 25]])
                nc.vector.tensor_scalar(out=dstq, in0=Y3, scalar1=8.0,
                                        scalar2=11.0, op0=ALU.max, op1=ALU.min)

            conv2_blk(0)
            for blk in range(1, NPAIR // 4):
                conv2_blk(blk)
                conv3_blk(blk - 1)
            conv3_blk(NPAIR // 4 - 1)

            # ---- fc1 staging: 4 SWDGE DMAs (src half g -> dest half jh) ----
            for g in range(2):
                for jh in range(2):
                    srcp = q3[64 * g:64 * g + 64, :]
                    src = bass.AP(tensor=srcp.tensor,
                                  offset=srcp.offset + jh * NPAIR,
                                  ap=list(srcp.ap[:1]) + [[2 * NPAIR, 14], [1, NPAIR]])
                    dstf = F[64 * jh:64 * jh + 64, :, :]
                    dst = bass.AP(tensor=dstf.tensor,
                                  offset=dstf.offset + ch * NB + g * NPAIR,
                                  ap=list(dstf.ap[:1]) + [[512, 14], [1, NPAIR]])
                    nc.sync.dma_start(out=dst, in_=src)

        # ---- fc1: 7 DoubleRow passes x 4 m-blocks ----
        for m in range(4):
            psft = ps_c2.tile([128, 4, 256], fp32, tag="c2")
            psf = psft.rearrange("p a b -> p (a b)")[:, 0:512]
            for k in range(7):
                nc.tensor.matmul(psf, FW1[:, 2 * k:2 * k + 2, 128 * m:128 * (m + 1)],
                                 F[:, 2 * k:2 * k + 2, :],
                                 start=(k == 0), stop=(k == 6), perf_mode=DR)
            nc.scalar.activation(out=H1[m], in_=psf, func=AF.Identity,
                                 bias=FB1[:, m:m + 1], scale=c13)

        # ---- fc2 (fc1 acts stationary) + log_softmax ----
        for qq in range(4):
            psjt = ps_c2.tile([128, 4, 256], fp32, tag="c2")
            pj = psjt[:, 0, 0:10]
            for k2 in range(4):
                nc.tensor.matmul(pj, H1[k2][:, 128 * qq:128 * (qq + 1)],
                                 FW2[:, k2, :], start=(k2 == 0), stop=(k2 == 3))
            v = tmp.tile([128, 10], fp32, tag="lg")
            nc.vector.tensor_add(v, pj, FB2T)
            mx = tmp.tile([128, 1], fp32, tag="mx")
            nc.vector.reduce_max(out=mx, in_=v, axis=AX)
            tt = tmp.tile([128, 10], fp32, tag="tt")
            nc.vector.tensor_scalar(out=tt, in0=v, scalar1=mx, scalar2=None,
                                    op0=ALU.subtract)
            ee = tmp.tile([128, 10], fp32, tag="ee")
            ss = tmp.tile([128, 1], fp32, tag="ss")
            nc.scalar.activation(out=ee, in_=tt, func=AF.Exp, accum_out=ss)
            ll = tmp.tile([128, 1], fp32, tag="ll")
            nc.scalar.activation(out=ll, in_=ss, func=AF.Ln)
            oo = tmp.tile([128, 10], fp32, tag="oo")
            nc.vector.tensor_scalar(out=oo, in0=tt, scalar1=ll, scalar2=None,
                                    op0=ALU.subtract)
            nc.sync.dma_start(out=d_out[128 * qq:128 * (qq + 1), :], in_=oo)

    nc.finalize()
    return nc


_NC_CACHE = None
_JIT_CACHE = None


def _build_jit(nc):
    """Build the sharded jitted executor once (same lowering path as
    run_bass_kernel_spmd under axon, but cached across kernel() calls)."""
    import jax
    from jax.sharding import Mesh, PartitionSpec
    from jax.experimental.shard_map import shard_map
    import concourse.mybir as mybir
    from concourse import bass2jax as b2j

    b2j.install_neuronx_cc_hook()
    partition_name = (nc.partition_id_tensor.name
                      if nc.partition_id_tensor else None)
    in_names, out_names, out_avals, zero_shapes = [], [], [], []
    for alloc in nc.m.functions[0].allocations:
        if not isinstance(alloc, mybir.MemoryLocationSet):
            continue
        name = alloc.memorylocations[0].name
        if alloc.kind == "ExternalInput":
            if name != partition_name:
                in_names.append(name)
        elif alloc.kind == "ExternalOutput":
            out_names.append(name)
            shape = tuple(alloc.tensor_shape)
            dtype = mybir.dt.np(alloc.dtype)
            out_avals.append(jax.core.ShapedArray(shape, dtype))
            zero_shapes.append((shape, dtype))
    n_params = len(in_names)
    n_outs = len(out_avals)
    in_names_all = (list(in_names) + out_names
                    + ([partition_name] if partition_name else []))
    donate = tuple(range(n_params, n_params + n_outs))

    def _body(*args):
        operands = list(args)
        if partition_name is not None:
            operands.append(b2j.partition_id_tensor())
        outs = b2j._bass_exec_p.bind(
            *operands, out_avals=tuple(out_avals),
            in_names=tuple(in_names_all), out_names=tuple(out_names),
            lowering_input_output_aliases=(), sim_require_finite=True,
            sim_require_nnan=True, nc=nc)
        return tuple(outs)

    devices = jax.devices()[:N_CORES]
    mesh = Mesh(np.asarray(devices), ("core",))
    sharded = jax.jit(
        shard_map(_body, mesh=mesh,
                  in_specs=(PartitionSpec("core"),) * (n_params + n_outs),
                  out_specs=(PartitionSpec("core"),) * n_outs,
                  check_rep=False),
        donate_argnums=donate, keep_unused=True)
    return sharded, in_names, out_names, zero_shapes


def kernel(**inputs):
    global _NC_CACHE, _JIT_CACHE
    x = np.asarray(inputs["x"], dtype=np.float32).reshape(4096, 28, 28)
    consts = _prep(**{k: v for k, v in inputs.items() if k != "x"})

    if _NC_CACHE is None:
        _NC_CACHE = _build_nc()
    if _JIT_CACHE is None:
        _JIT_CACHE = _build_jit(_NC_CACHE)
    sharded, in_names, out_names, zero_shapes = _JIT_CACHE

    ones = np.ones(NPAIR * 840, dtype=F16)
    per_core = []
    for corei in range(N_CORES):
        t0, t1 = _split_x(x[corei * B_CORE:(corei + 1) * B_CORE])
        m = dict(s0=t0, s1=t1, ones=ones)
        m.update(consts)
        per_core.append(m)
    concat_in = [np.concatenate([np.asarray(m[n]) for m in per_core], axis=0)
                 for n in in_names]
    concat_zeros = [np.zeros((N_CORES * s[0], *s[1:]), d)
                    for (s, d) in zero_shapes]
    outs = sharded(*concat_in, *concat_zeros)
    o = np.asarray(outs[out_names.index("out")])
    return o.reshape(N_CORES * B_CORE, 10).astype(np.float32)
